# revision 13
# baseline (speedup 1.0000x reference)
"""Trainium2 Bass kernel for ConstrainedProbabilityMatrixFactorization.

rating = uw @ iw.T + ub + ib.T + bias + (fb_values . E[fb_indices]) @ iw.T

Split as two PSUM-accumulation passes per 512-col bank:
  pass 1 (f32):  psum[n]  = ueT.T @ rhs_aug[:, n]     ueT = [uw | ub+bias | 1].T
  pass 2 (bf16): psum[n] += offsT.T @ iw.T[:, n]      offs = einsum(fb_values, E[fb_indices])

Sharding: the 1024-user batch is split across 8 NeuronCores (128 users
per core). No collectives.

The dominant cost is SWDGE descriptor generation for the feedback
segment-gather (~7ns/descriptor, 6400 descriptors/core). The Q7 kernel
for dma_gather only uses the core pair selected by queue_num
(cpu_id/2 == queue_num), so the gather is split into 8 chunks over
queue_nums 0-3: 4 descriptor generators run concurrently, two waves
each (front-loaded so the last wave's drain tail is short).

The gather reads a PAIRED bf16 view of item_rating_effect_weight
[25000, 128] (idx = row//2 fits int16, 256B/descriptor). Row parity is
resolved by host-built expanded weights w2x[p, (2l+parity)*64 + d] =
fb_values[p, l] (other half-slot zero), so the weighted segment-sum is
8 flat contiguous bf16 multiplies + 2 contiguous 7-op add-trees (one
per 25-slot wave) -- no strided reduces, no broadcast row overhead.

offs -> lhsT via HWDGE dma_start_transpose (bf16 [128,128]).
Base pass 1 runs on the PE while the gathers generate descriptors;
only pass 2 (+ PSUM copy-out) trails the segment reduce.
"""

import numpy as np

N_USERS = 100000
N_ITEMS = 50000
NPAIR = N_ITEMS // 2       # 25000 paired rows; index fits int16
D = 64
D2 = 2 * D                 # 128: paired row width (elems)
BU = 1024
BI = 4096
L = 50
NCORES = 8
UB = BU // NCORES          # 128 users per core
P = 128
K = D + 2                  # 66: augmented contraction dim (pass 1)
NBANK = 8                  # output column blocks of 512
# chunk i covers slots [CH_OFF[i], CH_OFF[i] + CH_N[i]). The 8 DMASW sem
# lanes are assigned round-robin over Pool-DMA instructions and each lane
# locks to one SWDGE queue, so the queue pattern must repeat mod 4 to keep
# every lane on a single queue. Three waves per queue, front-loaded
# (7/7/6/6 then 4/4/4/4 then 2/2/2/2): descriptor DRAIN only starts when a
# gather's generation completes, so later waves overlap earlier drains and
# the tiny last wave keeps the final drain tail short.
CH_N = [6, 7, 6, 6, 4, 4, 4, 4, 3, 2, 2, 2]
CH_OFF = [0, 6, 13, 19, 25, 29, 33, 37, 41, 44, 46, 48]
CH_Q = [0, 1, 2, 3, 0, 1, 2, 3, 0, 1, 2, 3]

_cached = {}


def _build_program():
    import concourse.bacc as bacc
    import concourse.bass as bass
    import concourse.mybir as mybir
    import concourse.tile as tile

    f32 = mybir.dt.float32
    bf16 = mybir.dt.bfloat16
    i16 = mybir.dt.int16

    nc = bacc.Bacc(num_swdge_queues=4)

    idx_in = [
        nc.dram_tensor(f"idx{i}", [P, CH_N[i] * 8], i16, kind="ExternalInput")
        for i in range(len(CH_N))
    ]
    w2_in = nc.dram_tensor("w2", [P, 2 * L], f32, kind="ExternalInput")
    ueT_in = nc.dram_tensor("ueT", [K, P], bf16, kind="ExternalInput")
    ereP = nc.dram_tensor("ereP", [NPAIR, D2], bf16, kind="ExternalInput")
    rhs_in = nc.dram_tensor("rhs16", [K, BI], bf16, kind="ExternalInput")
    rating = nc.dram_tensor("rating", [UB, BI], f32, kind="ExternalOutput")

    with tile.TileContext(nc) as tc:
        with (
            tc.tile_pool(name="sb", bufs=1) as sb,
            tc.tile_pool(name="sb_out", bufs=4) as sb_out,
            tc.tile_pool(name="ps", bufs=1, space="PSUM") as ps,
        ):
            # --- index tiles, then the chunked gathers (the ~9.4us mlp
            # ext-isa IRAM load is tied to the auto-inserted ModifyPoolConfig
            # at program start, so the first gather can begin right when it
            # finishes) ---
            idx_s = []
            for i in range(len(CH_N)):
                t = sb.tile([P, CH_N[i] * 8], i16, tag=f"idx{i}")
                nc.sync.dma_start(out=t[:], in_=idx_in[i][:])
                idx_s.append(t)

            gp = sb.tile([P, L * D2], bf16)   # [128, 50, 128] paired rows
            for i in range(len(CH_N)):
                n = CH_N[i] * P
                nc.gpsimd.dma_gather(
                    out_ap=gp[
                        :, CH_OFF[i] * D2 : (CH_OFF[i] + CH_N[i]) * D2
                    ].rearrange("p (l e) -> p l e", e=D2),
                    in_ap=ereP[:],
                    idxs_ap=idx_s[i][:],
                    num_idxs=n,
                    num_idxs_reg=n,
                    elem_size=D2,
                    single_packet=True,
                    queue_num=CH_Q[i],
                )

            # --- streaming loads (kept small: the gather drain shares the
            # SDMA engines, so input bytes directly delay the segment data) ---
            w2_s = sb.tile([P, 2 * L], f32)
            nc.sync.dma_start(out=w2_s[:], in_=w2_in[:])
            ueT = sb.tile([K, P], bf16)
            nc.sync.dma_start(out=ueT[:], in_=ueT_in[:])
            rhs = sb.tile([K, BI], bf16)
            nc.sync.dma_start(out=rhs[:], in_=rhs_in[:])

            # expand w2 [128, 100] -> w2x [128, 100, 64] bf16 on the Act
            # engine during the gather window (broadcast copy; Act is idle
            # here, and unlike the DVE it does not share an SBUF port with
            # the Q7 descriptor generators)
            w2x_s = sb.tile([P, L * D2], bf16)
            nc.scalar.copy(
                out=w2x_s[:].rearrange("p (s d) -> p s d", d=D),
                in_=w2_s[:].to_broadcast([P, 2 * L, D]),
            )

            # --- pass 1: base rating, overlapped with the gathers ---
            psum_all = ps.tile([P, NBANK * 512], f32, space="PSUM", tag="mm")
            psts = [psum_all[:, n * 512 : (n + 1) * 512] for n in range(NBANK)]
            for n in range(NBANK):
                nc.tensor.matmul(
                    out=psts[n],
                    lhsT=ueT[:],
                    rhs=rhs[:, n * 512 : (n + 1) * 512],
                    start=True,
                    stop=False,
                )

            # --- weighted products (flat contiguous bf16) interleaved with
            # the two half-trees: wave-1 chunks cover slots [0:26) so the
            # first tree (slots [0:25)) runs while waves 2-3 drain ---
            prod = sb.tile([P, L * D2], bf16)
            accs = []
            for w in range(2):
                t = sb.tile([P, P], bf16, tag=f"acc{w}")
                nc.vector.memset(t[:], 0.0)
                accs.append(t)

            def _mults(lo, hi):
                for i in range(lo, hi):
                    a, b = CH_OFF[i] * D2, (CH_OFF[i] + CH_N[i]) * D2
                    nc.vector.tensor_tensor(
                        out=prod[:, a:b],
                        in0=gp[:, a:b],
                        in1=w2x_s[:, a:b],
                        op=mybir.AluOpType.mult,
                    )

            for w in range(2):
                _mults(4 * w, 4 * w + 4)
                if w == 1:
                    _mults(8, 12)
                base = w * 25 * D2  # 3200 elems: 50 half-slots
                A = sb.tile([P, 25 * D], bf16, tag=f"treeA{w}")
                nc.vector.tensor_tensor(
                    out=A[:], in0=prod[:, base : base + 1600],
                    in1=prod[:, base + 1600 : base + 3200],
                    op=mybir.AluOpType.add,
                )
                B = sb.tile([P, 12 * D], bf16, tag=f"treeB{w}")
                nc.vector.tensor_tensor(
                    out=B[:], in0=A[:, 0:768], in1=A[:, 768:1536],
                    op=mybir.AluOpType.add,
                )
                C = sb.tile([P, 6 * D], bf16, tag=f"treeC{w}")
                nc.vector.tensor_tensor(
                    out=C[:], in0=B[:, 0:384], in1=B[:, 384:768],
                    op=mybir.AluOpType.add,
                )
                Dm = sb.tile([P, 3 * D], bf16, tag=f"treeD{w}")
                nc.vector.tensor_tensor(
                    out=Dm[:], in0=C[:, 0:192], in1=C[:, 192:384],
                    op=mybir.AluOpType.add,
                )
                E = sb.tile([P, D], bf16, tag=f"treeE{w}")
                nc.vector.tensor_tensor(
                    out=E[:], in0=Dm[:, 0:64], in1=Dm[:, 64:128],
                    op=mybir.AluOpType.add,
                )
                F = sb.tile([P, D], bf16, tag=f"treeF{w}")
                nc.vector.tensor_tensor(
                    out=F[:], in0=E[:], in1=Dm[:, 128:192],
                    op=mybir.AluOpType.add,
                )
                nc.vector.tensor_tensor(
                    out=accs[w][:, 0:D], in0=F[:], in1=A[:, 1536:1600],
                    op=mybir.AluOpType.add,
                )
                # half-offs -> lhsT via HWDGE xbar transpose (bf16); the
                # w=0 transpose and its pass-2a matmuls hide under the
                # later gather waves
                oT = sb.tile([P, P], bf16, tag=f"offsT{w}")
                nc.sync.dma_start_transpose(out=oT[:], in_=accs[w][:])
                for n in range(NBANK):
                    nc.tensor.matmul(
                        out=psts[n],
                        lhsT=oT[0:D, :],
                        rhs=rhs[0:D, n * 512 : (n + 1) * 512],
                        start=False,
                        stop=(w == 1),
                    )

            # --- copy-out: wide copies spanning two PSUM banks each ---
            for h in range(4):
                ot = sb_out.tile([P, 1024], f32, tag="ot")
                if h % 2 == 0:
                    nc.scalar.copy(out=ot[:], in_=psum_all[:, h * 1024 : (h + 1) * 1024])
                else:
                    nc.vector.tensor_copy(out=ot[:], in_=psum_all[:, h * 1024 : (h + 1) * 1024])
                nc.sync.dma_start(
                    out=rating[:, h * 1024 : (h + 1) * 1024], in_=ot[:]
                )

    nc.finalize()
    return nc


def _get_program():
    if "nc" not in _cached:
        _cached["nc"] = _build_program()
    return _cached["nc"]


def _prep_inputs(inputs):
    user_ids = np.asarray(inputs["user_ids"]).astype(np.int64)
    item_ids = np.asarray(inputs["item_ids"]).astype(np.int64)
    fb_indices = np.asarray(inputs["fb_indices"]).astype(np.int64)
    fb_values = np.asarray(inputs["fb_values"]).astype(np.float32)
    uw = np.asarray(inputs["user_weight"], dtype=np.float32)
    ub = np.asarray(inputs["user_bias"], dtype=np.float32).reshape(N_USERS, 1)
    iw = np.asarray(inputs["item_weight"], dtype=np.float32)
    ib = np.asarray(inputs["item_bias"], dtype=np.float32).reshape(N_ITEMS, 1)
    ire = np.asarray(inputs["item_rating_effect_weight"], dtype=np.float32)
    bias = float(np.asarray(inputs["bias"], dtype=np.float32).reshape(-1)[0])

    # item batch: order known host-side; device streams it contiguously
    # rhs16 = [iw.T ; ones ; ib.T] in bf16
    rhs16 = np.empty((K, BI), dtype=np.float32)
    rhs16[0:D] = iw[item_ids].T
    rhs16[D] = 1.0
    rhs16[D + 1] = ib[item_ids, 0]
    rhs16 = _to_bf16(rhs16)

    ereP16 = _to_bf16(ire.reshape(NPAIR, D2))              # paired bf16 view

    in_maps = []
    for c in range(NCORES):
        sl = slice(c * UB, (c + 1) * UB)
        fbi_c = fb_indices[sl]                 # [128, 50]
        fbv_c = fb_values[sl]

        # ueT = [uw | ub+bias | 1].T for this core's users
        ue = np.empty((UB, K), dtype=np.float32)
        ue[:, 0:D] = uw[user_ids[sl]]
        ue[:, D] = ub[user_ids[sl], 0] + bias
        ue[:, D + 1] = 1.0
        ueT = _to_bf16(np.ascontiguousarray(ue.T))  # [66, 128] bf16

        # w2x[p, (2l+parity)*64 + d] = fbv[p, l]; other half-slot 0
        parity = (fbi_c & 1).astype(np.int64)  # [128, 50]
        w2 = np.zeros((UB, 2 * L), dtype=np.float32)
        rows = np.repeat(np.arange(UB), L)
        cols = (2 * np.arange(L)[None, :] + parity).reshape(-1)
        w2[rows, cols] = fbv_c.reshape(-1)

        # per-chunk dma_gather index tiles
        pair_all = (fbi_c >> 1).astype(np.int16)  # [128, 50]
        idx_tiles = []
        for i in range(len(CH_N)):
            n16 = CH_N[i] * 8                  # idx columns (num/16)
            flat = pair_all[:, CH_OFF[i] : CH_OFF[i] + CH_N[i]].T.reshape(-1)
            s_idx = (
                np.arange(n16)[None, :] * 16 + (np.arange(P) % 16)[:, None]
            )
            idx_tiles.append(np.ascontiguousarray(flat[s_idx]))

        m = {
            "w2": w2,
            "ueT": ueT,
            "ereP": ereP16,
            "rhs16": rhs16,
        }
        for i in range(len(CH_N)):
            m[f"idx{i}"] = idx_tiles[i]
        in_maps.append(m)
    return in_maps


def _to_bf16(a):
    import ml_dtypes

    return a.astype(ml_dtypes.bfloat16)


def run(inputs, trace=False):
    """Returns (output [1024, 4096] f32, BassKernelResults)."""
    from concourse import bass_utils

    nc = _get_program()
    in_maps = _prep_inputs(inputs)
    res = bass_utils.run_bass_kernel_spmd(
        nc, in_maps, core_ids=list(range(NCORES)), trace=trace
    )
    out = np.concatenate([res.results[c]["rating"] for c in range(NCORES)], axis=0)
    return out, res


def kernel(**inputs) -> np.ndarray:
    out, _ = run(inputs, trace=False)
    return out
